# revision 1
# baseline (speedup 1.0000x reference)
"""Trainium2 Bass kernel for FFNWithScales (SwiGLU MLP with low-rank dequant scales).

Reference computation (all fp32):
    gate_eff = gate_snapped * (gate_scale_A @ gate_scale_B)       # [8192, 2048]
    up_eff   = up_snapped   * (up_scale_A   @ up_scale_B)         # [8192, 2048]
    down_eff = down_snapped * (down_scale_A @ down_scale_B)       # [2048, 8192]
    h   = silu(gate_eff @ x) * (up_eff @ x)                       # [8192, 512]
    out = down_eff @ h                                            # [2048, 512]

Sharding (8 cores, tensor-parallel on d_ff): core c owns d_ff rows
[c*1024, (c+1)*1024) of gate/up (and the matching columns of down).
Each core computes a full-[2048, 512] partial of the down projection;
partials are summed on the host (the all-reduce step).

Device notes:
  - PE matmul computes psum[M,N] = lhsT[K,M].T @ rhs[K,N] with K on
    partitions, so every weight is fed with its contraction dim on
    partitions. The host pre-transposes the snapped weights (one numpy
    transpose each) because fp32 has no DMA-transpose path on TRN2.
  - The fp32 snapped weights (24 MiB/core — the dominant HBM traffic)
    stream through in [128, 2, 512] pairs: one 512 KiB DMA, a packed
    pair of rank-32 scale matmuls (row-tiled via tile_position so both
    run concurrently in the PE array), one DVE dequant multiply that
    rounds to bf16, then eight [128,128]x[128,512] bf16 main matmuls
    with fp32 psum accumulation. bf16 streams ~3x faster than fp32r on
    the PE, which is what makes the kernel DMA-bound. Measured
    end-to-end error vs the fp32 reference: ~5e-3 of output absmax.
  - The broadcast activations x and the rank-32 factors are shipped
    bf16 in their final device layouts (host prep), so no on-device
    staging/rounding chain exists to stall the weight pipeline.
  - DMA rings: sync HWDGE carries only the weight stream (HWDGE is
    FIFO per issuing engine — a waiting DMA would head-of-line block
    the stream), scalar HWDGE carries the small constant loads, and
    output stores go out the gpsimd SWDGE ring.
  - Each pass's first scale-pack/dequant is emitted before the
    previous pass's epilogue so pass boundaries only wait on psum
    accumulator release.
"""

import numpy as np
import ml_dtypes

import concourse.bass as bass
from concourse import bacc
import concourse.mybir as mybir
from concourse.tile import TileContext
from concourse.bass_utils import run_bass_kernel_spmd

P = 128
D = 2048        # d_model
FF = 8192       # d_ff (global)
S = 512         # sequence
R = 32          # rank
NCORES = 8
F = FF // NCORES          # 1024 local d_ff rows
KD = D // P               # 16 d_model chunks
KF = F // P               # 8 local d_ff chunks
FG = 512                  # free-dim group (psum bank width)

f32 = mybir.dt.float32
bf16 = mybir.dt.bfloat16

_CACHE = {}


def _build():
    nc = bacc.Bacc()
    # x / scale factors arrive bf16 in device layout; weights arrive fp32.
    x = nc.declare_dram_parameter("x", [D, S], bf16, isOutput=False)
    gT = nc.declare_dram_parameter("gT", [D, F], f32, isOutput=False)
    uT = nc.declare_dram_parameter("uT", [D, F], f32, isOutput=False)
    dT = nc.declare_dram_parameter("dT", [F, D], f32, isOutput=False)
    # B2 [64, nk/2, 128]: strip i holds B cols for kd-chunk 2*kp+i (lhsT of
    # the packed scale matmul); AT2 [64, w]: A^T replicated on both strips.
    gB2 = nc.declare_dram_parameter("gB2", [2 * R, KD // 2, P], bf16, isOutput=False)
    uB2 = nc.declare_dram_parameter("uB2", [2 * R, KD // 2, P], bf16, isOutput=False)
    dB2 = nc.declare_dram_parameter("dB2", [2 * R, KF // 2, P], bf16, isOutput=False)
    gAT2 = nc.declare_dram_parameter("gAT2", [2 * R, F], bf16, isOutput=False)
    uAT2 = nc.declare_dram_parameter("uAT2", [2 * R, F], bf16, isOutput=False)
    dAT2 = nc.declare_dram_parameter("dAT2", [2 * R, D], bf16, isOutput=False)
    out = nc.declare_dram_parameter("out", [D, S], f32, isOutput=True)

    with TileContext(nc) as tc:
        with (
            tc.tile_pool(name="const", bufs=1) as const,
            tc.tile_pool(name="wstream", bufs=14) as wpool,
            tc.tile_pool(name="hbuf", bufs=1) as hpool,
            tc.tile_pool(name="obuf", bufs=3) as opool,
            tc.tile_pool(name="psacc", bufs=1, space="PSUM") as psacc,
            tc.tile_pool(name="pssc", bufs=2, space="PSUM") as pssc,
        ):
            # Startup critical path: the first scale-pack needs the gate
            # factors and the first mains need x chunk 0, so those lead the
            # sync ring right before the weight stream; everything else
            # loads on the scalar ring.
            rounded = {}

            def load_factor(nm, dram, eng):
                rt = const.tile(list(dram.shape), bf16, name=f"{nm}r", tag=f"{nm}r")
                eng.dma_start(rt, dram[:])
                rounded[nm] = rt

            load_factor("gB", gB2, nc.sync)
            load_factor("gAT", gAT2, nc.scalar)

            XC = 2
            x_sb = [None] * (KD // XC)

            def load_x_chunk(q, eng):
                xt = const.tile([P, XC, S], bf16, name=f"x{q}", tag=f"x{q}")
                eng.dma_start(
                    xt, x[q * XC * P:(q + 1) * XC * P, :].rearrange(
                        "(ko p) s -> p ko s", p=P))
                x_sb[q] = xt

            def xs(kd):
                return x_sb[kd // XC][:, kd % XC]

            load_x_chunk(0, nc.sync)
            load_x_chunk(1, nc.scalar)

            load_factor("uB", uB2, nc.gpsimd)
            load_factor("uAT", uAT2, nc.gpsimd)
            load_factor("dBs", dB2, nc.gpsimd)
            load_factor("dAT", dAT2, nc.gpsimd)

            # h = silu(gate) * up, [128, 8, 512] resident
            h_sb = hpool.tile([P, KF, S], bf16)

            silu = mybir.ActivationFunctionType.Silu

            def gate_up_finish(acc, fg, is_up):
                for fi in range(4):
                    f = fg * 4 + fi
                    if is_up:
                        nc.vector.tensor_mul(
                            out=h_sb[:, f], in0=h_sb[:, f], in1=acc[fi])
                    else:
                        nc.scalar.activation(h_sb[:, f], acc[fi], silu)

            def down_finish(acc, mg):
                if mg < D // FG - 1:
                    # two batched [128, 2, 512] stores; the copy runs right
                    # before its store at the same program position, so the
                    # HWDGE store can't head-of-line block the remaining
                    # weight stream for long.
                    for half in range(2):
                        ot2 = opool.tile([P, 2, S], f32, name="ot", tag="ot")
                        for j in range(2):
                            nc.scalar.copy(ot2[:, j], acc[half * 2 + j])
                        weng = nc.sync if half == 0 else nc.scalar
                        weng.dma_start(
                            out[(mg * 4 + half * 2) * P:
                                (mg * 4 + half * 2 + 2) * P, :].rearrange(
                                "(mo p) s -> p mo s", p=P), ot2)
                else:
                    # final pass: this epilogue is the kernel tail, so drain
                    # it wide — copies split across ACT and DVE (both idle by
                    # now), four small stores across both HWDGE rings.
                    for mi in range(4):
                        ot = opool.tile([P, S], f32, name="otl", tag="otl")
                        if mi % 2 == 0:
                            nc.scalar.copy(ot, acc[mi])
                        else:
                            nc.vector.tensor_copy(out=ot, in_=acc[mi])
                        weng = nc.sync if mi % 2 == 0 else nc.scalar
                        weng.dma_start(
                            out[(mg * 4 + mi) * P:(mg * 4 + mi + 1) * P, :],
                            ot)

            passes = []
            for is_up in (0, 1):
                for fg in range(F // FG):
                    passes.append(dict(
                        wdram=uT if is_up else gT,
                        Bn="uB" if is_up else "gB",
                        An="uAT" if is_up else "gAT",
                        nk=KD, fg=fg, rhs_fn=xs,
                        finish=lambda acc, fg=fg, is_up=is_up:
                            gate_up_finish(acc, fg, is_up),
                    ))
            for mg in range(D // FG):
                passes.append(dict(
                    wdram=dT, Bn="dBs", An="dAT",
                    nk=KF, fg=mg, rhs_fn=lambda kf: h_sb[:, kf],
                    finish=lambda acc, mg=mg: down_finish(acc, mg),
                ))

            sc_tiles = {}

            def emit_sc(pi, kp):
                ps = passes[pi]
                fg = ps["fg"]
                sc2 = pssc.tile([P, 2, FG], f32, name="sc", tag="sc")
                for i in range(2):
                    nc.tensor.matmul(
                        sc2[:, i],
                        rounded[ps["Bn"]][i * R:(i + 1) * R, kp],
                        rounded[ps["An"]][i * R:(i + 1) * R,
                                          fg * FG:(fg + 1) * FG],
                        start=True, stop=True,
                        tile_position=(R * i, 0),
                    )
                sc_tiles[pi, kp] = sc2

            wr_tiles = {}

            wt_tiles = {}

            def emit_wt(pi, kp):
                """Weight DMA for pair (pi, kp). Issued several jobs ahead
                of the dequant (no psum involved) so a slow transfer can't
                starve the PE."""
                ps = passes[pi]
                fg = ps["fg"]
                wt2 = wpool.tile([P, 2, FG], f32, name="wt", tag="wt")
                # alternate the weight stream across both HWDGE rings so
                # back-to-back pairs overlap their transfer+completion
                # latency instead of serializing on one ring.
                weng = nc.sync if (pi + kp) % 2 == 0 else nc.scalar
                weng.dma_start(
                    wt2,
                    ps["wdram"][kp * 2 * P:(kp + 1) * 2 * P,
                                fg * FG:(fg + 1) * FG].rearrange(
                                    "(ko p) f -> p ko f", p=P))
                wt_tiles[pi, kp] = wt2

            def emit_dequant(pi, kp):
                wr2 = wpool.tile([P, 2, FG], bf16, name="wr", tag="wr")
                nc.vector.tensor_mul(out=wr2, in0=wt_tiles.pop((pi, kp)),
                                     in1=sc_tiles.pop((pi, kp)))
                wr_tiles[pi, kp] = wr2

            # Flat pair-job list across all passes, software-pipelined with
            # TWO pairs of scale-pack/dequant look-ahead (2 sc psum slots
            # cover it: job J+2's pack allocates the slot job J's dequant
            # just freed). The deeper pipeline absorbs DVE timing jitter at
            # pair and pass boundaries.
            jobs = []
            for pi, ps in enumerate(passes):
                for kp in range(ps["nk"] // 2):
                    jobs.append((pi, kp))
            LOOKAHEAD = 2
            DMA_AHEAD = 6
            for J in range(DMA_AHEAD):
                emit_wt(*jobs[J])
            for J in range(LOOKAHEAD):
                emit_sc(*jobs[J])
                emit_dequant(*jobs[J])

            cur_acc = {}
            for J, (pi, kp) in enumerate(jobs):
                ps = passes[pi]
                npairs = ps["nk"] // 2
                if kp == 0:
                    cur_acc[pi] = [
                        psacc.tile([P, S], f32, name=f"acc{i}", tag=f"acc{i}")
                        for i in range(4)]
                if pi == 0 and 2 <= kp + 2 < KD // XC:
                    # pull the rest of x in just-in-time on the ring the
                    # weight stream isn't using this iteration (chunk q
                    # is first consumed at pair kp=q).
                    load_x_chunk(
                        kp + 2,
                        nc.scalar if (pi + kp) % 2 == 0 else nc.sync)
                if J + DMA_AHEAD < len(jobs):
                    emit_wt(*jobs[J + DMA_AHEAD])
                if J + LOOKAHEAD < len(jobs):
                    emit_sc(*jobs[J + LOOKAHEAD])
                    emit_dequant(*jobs[J + LOOKAHEAD])
                acc = cur_acc[pi]
                wr2 = wr_tiles.pop((pi, kp))
                for j in range(2):
                    for fi in range(4):
                        nc.tensor.matmul(
                            acc[fi],
                            wr2[:, j, fi * P:(fi + 1) * P],
                            ps["rhs_fn"](2 * kp + j),
                            start=(kp == 0 and j == 0),
                            stop=(kp == npairs - 1 and j == 1),
                        )
                if kp == npairs - 1:
                    ps["finish"](cur_acc.pop(pi))
    nc.finalize()
    return nc


def _prep_inputs(x, gate_snapped, gate_scale_A, gate_scale_B,
                 up_snapped, up_scale_A, up_scale_B,
                 down_snapped, down_scale_A, down_scale_B):
    asf = lambda a: np.ascontiguousarray(np.asarray(a, dtype=np.float32))
    bf = ml_dtypes.bfloat16
    x2 = np.ascontiguousarray(np.asarray(x, dtype=np.float32).reshape(D, S)
                              .astype(bf))
    gT_full = asf(gate_snapped).T      # [D, FF] view
    uT_full = asf(up_snapped).T
    dT_full = asf(down_snapped).T      # [FF, D] view

    def pack_B2(Bmat, nk):
        # [R, nk*128] -> [64, nk/2, 128]: strip i holds chunks 2*kp+i
        b = np.asarray(Bmat, dtype=np.float32).reshape(R, nk // 2, 2, P)
        o = np.empty((2 * R, nk // 2, P), dtype=bf)
        o[:R] = b[:, :, 0, :].astype(bf)
        o[R:] = b[:, :, 1, :].astype(bf)
        return o

    def pack_AT2(Amat):
        # A [w, R] -> A^T [R, w] replicated on both strips -> [64, w]
        at = np.asarray(Amat, dtype=np.float32).T.astype(bf)
        return np.ascontiguousarray(np.concatenate([at, at], axis=0))

    gB_f = np.asarray(gate_scale_B, dtype=np.float32)
    uB_f = np.asarray(up_scale_B, dtype=np.float32)
    dB_f = np.asarray(down_scale_B, dtype=np.float32)
    gA_f = np.asarray(gate_scale_A, dtype=np.float32)
    uA_f = np.asarray(up_scale_A, dtype=np.float32)
    dAT2 = pack_AT2(down_scale_A)      # [64, D]

    in_maps = []
    for c in range(NCORES):
        lo, hi = c * F, (c + 1) * F
        in_maps.append({
            "x": x2,
            "gT": np.ascontiguousarray(gT_full[:, lo:hi]),
            "uT": np.ascontiguousarray(uT_full[:, lo:hi]),
            "dT": np.ascontiguousarray(dT_full[lo:hi, :]),
            "gB2": pack_B2(gB_f, KD),
            "uB2": pack_B2(uB_f, KD),
            "dB2": pack_B2(dB_f[:, lo:hi], KF),
            "gAT2": pack_AT2(gA_f[lo:hi]),
            "uAT2": pack_AT2(uA_f[lo:hi]),
            "dAT2": dAT2,
        })
    return in_maps


def run(trace=False, **inputs):
    if "nc" not in _CACHE:
        _CACHE["nc"] = _build()
    nc = _CACHE["nc"]
    in_maps = _prep_inputs(**inputs)
    try:
        res = run_bass_kernel_spmd(nc, in_maps, list(range(NCORES)), trace=trace)
    except Exception:
        # A transient device flake (NRT_EXEC_UNIT_UNRECOVERABLE) poisons the
        # PJRT client for the process; tearing the backend down and
        # reconnecting recovers it the same way a fresh process does.
        try:
            import jax.extend.backend
            jax.extend.backend.clear_backends()
        except Exception:
            pass
        res = run_bass_kernel_spmd(nc, in_maps, list(range(NCORES)), trace=trace)
    partial = np.zeros((D, S), dtype=np.float32)
    for c in range(NCORES):
        partial += res.results[c]["out"]
    return partial.reshape(1, D, 1, S), res


def kernel(**inputs):
    out, _ = run(trace=False, **inputs)
    return out


if __name__ == "__main__":
    rng = np.random.default_rng(0)
    ins = {
        "x": rng.standard_normal((1, D, 1, S)).astype(np.float32),
        "gate_snapped": (rng.standard_normal((FF, D)) * 0.02).astype(np.float32),
        "gate_scale_A": (rng.standard_normal((FF, R)) * 0.1).astype(np.float32),
        "gate_scale_B": (rng.standard_normal((R, D)) * 0.1).astype(np.float32),
        "up_snapped": (rng.standard_normal((FF, D)) * 0.02).astype(np.float32),
        "up_scale_A": (rng.standard_normal((FF, R)) * 0.1).astype(np.float32),
        "up_scale_B": (rng.standard_normal((R, D)) * 0.1).astype(np.float32),
        "down_snapped": (rng.standard_normal((D, FF)) * 0.02).astype(np.float32),
        "down_scale_A": (rng.standard_normal((D, R)) * 0.1).astype(np.float32),
        "down_scale_B": (rng.standard_normal((R, FF)) * 0.1).astype(np.float32),
    }
    out = kernel(**ins)
    print("kernel ran, out shape", out.shape, "mean abs", np.abs(out).mean())



# revision 8
# speedup vs baseline: 1.0435x; 1.0435x over previous
"""Trainium2 Bass kernel for FFNWithScales (SwiGLU MLP with low-rank dequant scales).

Reference computation (all fp32):
    gate_eff = gate_snapped * (gate_scale_A @ gate_scale_B)       # [8192, 2048]
    up_eff   = up_snapped   * (up_scale_A   @ up_scale_B)         # [8192, 2048]
    down_eff = down_snapped * (down_scale_A @ down_scale_B)       # [2048, 8192]
    h   = silu(gate_eff @ x) * (up_eff @ x)                       # [8192, 512]
    out = down_eff @ h                                            # [2048, 512]

Sharding (8 cores, tensor-parallel on d_ff): core c owns d_ff rows
[c*1024, (c+1)*1024) of gate/up (and the matching columns of down).
Each core computes a full-[2048, 512] partial of the down projection;
partials are summed on the host (the all-reduce step).

Kernel design (v2 — PE-bound, so everything serves the PE stream):
  - All tensors ship bf16 from the host (snapped weights included): the
    extra bf16 rounding of snapped costs ~1e-3 relative error against a
    2e-2 budget, and it halves HBM traffic so DMA (~17 MB @ ~300 GB/s)
    stays far under the PE streaming time.
  - Weights are pre-transposed on host so their contraction dim rides the
    partitions, and are DMA'd in full-row tiles (2-4 KB contiguous per
    partition line): gate/up as [128 d, 1024 f] per d-chunk, down as
    [128 f, 2048 d] per f-chunk.
  - The rank-32 scale products run 4-way row-packed on the PE
    (tile_position strips 0/32/64/96): one ~280 ns stream covers two
    d-chunks' worth of scale tiles. The DVE dequant-multiplies the bf16
    snapped tile by the fp32 psum scale tile, emitting the bf16 wr tile
    the main matmuls consume. Dequanted gate/up/down weights stay
    resident in SBUF so each of the 48 dequants serves two passes.
  - Main matmuls are kd-major in the first pass (matches the dequant
    feed rate) and fi-major-blocked afterwards, so a psum accumulator's
    epilogue (silu / up-multiply / output copy) always drains behind
    12+ matmuls on other banks — pass boundaries never stall the PE.
  - Six dummy warm-up matmuls on a memset tile run while the first DMAs
    land, so the PE HAM clock-gate reaches 2.4 GHz before real work.
  - Output partials store bf16 (host accumulates in fp32), with the
    final pass's stores split across both HWDGE rings for a short tail.
"""

import numpy as np
import ml_dtypes

import concourse.bass as bass
from concourse import bacc
import concourse.mybir as mybir
from concourse.tile import TileContext
from concourse.bass_utils import run_bass_kernel_spmd

P = 128
D = 2048        # d_model
FF = 8192       # d_ff (global)
S = 512         # sequence
R = 32          # rank
NCORES = 8
F = FF // NCORES          # 1024 local d_ff rows
KD = D // P               # 16 d_model chunks
KF = F // P               # 8 local d_ff chunks

f32 = mybir.dt.float32
bf16 = mybir.dt.bfloat16

_CACHE = {}


def _build():
    nc = bacc.Bacc()
    x = nc.declare_dram_parameter("x", [D, S], bf16, isOutput=False)
    # snapped weights, transposed, tiled so a dram slice is an SBUF tile
    gT = nc.declare_dram_parameter("gT", [KD, P, 2, 512], bf16, isOutput=False)
    uT = nc.declare_dram_parameter("uT", [KD, P, 2, 512], bf16, isOutput=False)
    dT = nc.declare_dram_parameter("dT", [KF, P, 4, 512], bf16, isOutput=False)
    # 4-way packed scale factors: B strips for chunk pairs (both fg copies),
    # A^T replicated on all four 32-row strips.
    gB4 = nc.declare_dram_parameter("gB4", [4 * R, KD // 2, P], bf16, isOutput=False)
    uB4 = nc.declare_dram_parameter("uB4", [4 * R, KD // 2, P], bf16, isOutput=False)
    dB4 = nc.declare_dram_parameter("dB4", [4 * R, KF // 2, P], bf16, isOutput=False)
    gAT4 = nc.declare_dram_parameter("gAT4", [4 * R, F], bf16, isOutput=False)
    uAT4 = nc.declare_dram_parameter("uAT4", [4 * R, F], bf16, isOutput=False)
    dAT4 = nc.declare_dram_parameter("dAT4", [4 * R, D], bf16, isOutput=False)
    out = nc.declare_dram_parameter("out", [D, S], bf16, isOutput=True)

    silu = mybir.ActivationFunctionType.Silu

    with TileContext(nc) as tc:
        with (
            tc.tile_pool(name="const", bufs=1) as const,
            tc.tile_pool(name="wtg", bufs=6) as wtg,
            tc.tile_pool(name="wtd", bufs=8) as wtd,
            tc.tile_pool(name="gwr", bufs=1) as gwr,
            tc.tile_pool(name="uwr", bufs=1) as uwr,
            tc.tile_pool(name="dwr", bufs=1) as dwr,
            tc.tile_pool(name="hbuf", bufs=1) as hpool,
            tc.tile_pool(name="scb", bufs=3) as scbp,
            tc.tile_pool(name="obuf", bufs=3) as opool,
            tc.tile_pool(name="psacc", bufs=1, space="PSUM") as psacc,
            tc.tile_pool(name="pssc", bufs=2, space="PSUM") as pssc,
        ):
            # ---- constant loads (factors lead the rings, x0/x1 next) ----
            rounded = {}

            def load_const(nm, dram, eng):
                rt = const.tile(list(dram.shape), bf16, name=nm, tag=nm)
                eng.dma_start(rt, dram[:])
                rounded[nm] = rt

            load_const("gB4", gB4, nc.sync)
            load_const("gAT4", gAT4, nc.sync)

            x_sb = [None] * (KD // 2)

            def load_x_chunk(q, eng):
                xt = const.tile([P, 2, S], bf16, name=f"x{q}", tag=f"x{q}")
                eng.dma_start(
                    xt, x[q * 2 * P:(q + 1) * 2 * P, :].rearrange(
                        "(ko p) s -> p ko s", p=P))
                x_sb[q] = xt

            def xs(kd):
                return x_sb[kd // 2][:, kd % 2]

            load_x_chunk(0, nc.sync)
            load_x_chunk(1, nc.scalar)
            load_const("uB4", uB4, nc.gpsimd)
            load_const("uAT4", uAT4, nc.gpsimd)
            load_const("dB4", dB4, nc.gpsimd)
            load_const("dAT4", dAT4, nc.gpsimd)

            # ---- PE warm-up: dummy matmuls while the first DMAs land ----
            junk = const.tile([P, 640], bf16, name="junk", tag="junk")
            nc.vector.memset(junk, 0.0)
            for _ in range(6):
                wps = pssc.tile([P, 2, S], f32, name="sc", tag="sc")
                nc.tensor.matmul(wps[:, 0], junk[:, 0:128], junk[:, 128:640],
                                 start=True, stop=True)

            # ---- weight stream: 40 snapped-tile DMAs ----
            # jobs 0..15 gate kd, 16..31 up kd, 32..39 down kf
            wt_tiles = {}
            dma_parity = [0]

            def ring():
                dma_parity[0] ^= 1
                return nc.sync if dma_parity[0] else nc.scalar

            def emit_wt(j):
                if j < 16:
                    t = wtg.tile([P, 2, 512], bf16, name="wt", tag="wt")
                    ring().dma_start(t, gT[j])
                elif j < 32:
                    t = wtg.tile([P, 2, 512], bf16, name="wt", tag="wt")
                    ring().dma_start(t, uT[j - 16])
                else:
                    t = wtd.tile([P, 4, 512], bf16, name="wtd", tag="wtd")
                    ring().dma_start(t, dT[j - 32])
                wt_tiles[j] = t

            # ---- scale bursts + copies + dequants ----
            # dq jobs: 0..15 gate kd, 16..31 up kd, 32..47 down (h*8 + kf)
            # burst b covers dq jobs 2b, 2b+1. Each job is a 3-engine chain:
            # PE scale-matmul -> ACT psum->sbuf bf16 copy -> DVE bf16 2x
            # dequant multiply (all-16-bit keeps the DVE in its fast mode).
            sc_tiles = {}
            scb_tiles = {}
            dwr_tiles = {}
            dq_next = [0]
            wt_next = [0]

            def ensure_wt(upto):
                while wt_next[0] <= min(upto, 39):
                    emit_wt(wt_next[0])
                    wt_next[0] += 1

            def emit_burst(b):
                sc_a = pssc.tile([P, 2, S], f32, name="sc", tag="sc")
                sc_b = pssc.tile([P, 2, S], f32, name="sc", tag="sc")
                if b < 8:          # gate kd pair (2b, 2b+1)
                    Bm, Am, j = rounded["gB4"], rounded["gAT4"], b
                    cols = [0, 512, 0, 512]
                elif b < 16:       # up kd pair
                    Bm, Am, j = rounded["uB4"], rounded["uAT4"], b - 8
                    cols = [0, 512, 0, 512]
                else:              # down: m = b-16: h = m//4, kf pair j = m%4
                    m = b - 16
                    h = m // 4
                    Bm, Am, j = rounded["dB4"], rounded["dAT4"], m % 4
                    cols = [h * 1024, h * 1024 + 512, h * 1024, h * 1024 + 512]
                for i, dst in enumerate((sc_a[:, 0], sc_a[:, 1],
                                         sc_b[:, 0], sc_b[:, 1])):
                    nc.tensor.matmul(
                        dst,
                        Bm[i * R:(i + 1) * R, j],
                        Am[i * R:(i + 1) * R, cols[i]:cols[i] + 512],
                        start=True, stop=True,
                        tile_position=(R * i, 0),
                    )
                sc_tiles[2 * b] = sc_a
                sc_tiles[2 * b + 1] = sc_b

            wr_of = {}

            def emit_copy(j):
                # free the sc psum slot quickly on the otherwise-idle ACT
                # (a few on DVE where ACT is the tight engine)
                scb = scbp.tile([P, 2, 512], bf16, name="scb", tag="scb")
                sc = sc_tiles.pop(j)
                if 16 <= j < 32 and j % 4 == 3:
                    nc.vector.tensor_copy(out=scb, in_=sc)
                else:
                    nc.scalar.copy(scb, sc)
                scb_tiles[j] = scb

            def emit_dq(j):
                sc = scb_tiles.pop(j)
                if j < 16:
                    wr = gwr.tile([P, 2, 512], bf16, name="gwr", tag=f"g{j}")
                    nc.vector.tensor_mul(out=wr, in0=wt_tiles.pop(j), in1=sc)
                    wr_of["g", j] = wr
                elif j < 32:
                    wr = uwr.tile([P, 2, 512], bf16, name="uwr", tag=f"u{j-16}")
                    nc.vector.tensor_mul(out=wr, in0=wt_tiles.pop(j), in1=sc)
                    wr_of["u", j - 16] = wr
                else:
                    m = j - 32
                    h, kf = m // 8, m % 8
                    if h == 0:
                        dwr_tiles[kf] = dwr.tile([P, 4, 512], bf16, name="dwr",
                                                 tag=f"d{kf}")
                    wr = dwr_tiles[kf]
                    wt = wt_tiles[32 + kf] if h == 0 else wt_tiles.pop(32 + kf)
                    nc.vector.tensor_mul(out=wr[:, 2 * h:2 * h + 2],
                                         in0=wt[:, 2 * h:2 * h + 2], in1=sc)
                    wr_of["d", kf] = wr

            def _wt_for_dq(j):
                return j if j < 32 else 32 + (j - 32) % 8

            def ensure_dq(upto):
                while dq_next[0] <= min(upto, 47):
                    jj = dq_next[0]
                    ensure_wt(_wt_for_dq(jj) + 4)
                    if jj % 2 == 0:
                        emit_burst(jj // 2)
                    emit_copy(jj)
                    emit_dq(jj)
                    dq_next[0] += 1

            # prime the pipeline
            ensure_wt(3)
            ensure_dq(1)

            h_sb = hpool.tile([P, KF, S], bf16)

            # ---- pass 0: gate fg0, kd-pair-major (dequant-feed limited;
            # psum banks switch every 2 matmuls instead of every 1) ----
            acc = {fi: psacc.tile([P, S], f32, name=f"acc{fi}", tag=f"acc{fi}")
                   for fi in range(4)}
            for kdp in range(KD // 2):
                ensure_dq(min(2 * kdp + 3, 15))
                if kdp <= 5:
                    load_x_chunk(kdp + 2, ring())
                for fi in range(4):
                    for kd in (2 * kdp, 2 * kdp + 1):
                        nc.tensor.matmul(
                            acc[fi], wr_of["g", kd][:, 0, fi * P:(fi + 1) * P],
                            xs(kd), start=(kd == 0), stop=(kd == KD - 1))
            for fi in range(4):
                nc.scalar.activation(h_sb[:, fi], acc[fi], silu)

            # ---- passes 1-3: gate fg1 / up fg0 / up fg1 ----
            # Full k-contiguous runs: 16 matmuls into ONE psum bank (the
            # b2b rate is 216 ns same-bank vs 259 ns cycling), epilogue of
            # acc[fi] drains behind acc[fi+1]'s run.
            def gu_pass(mat, fg, dq_for_fi, epi):
                a = {fi: psacc.tile([P, S], f32, name=f"acc{fi}",
                                    tag=f"acc{fi}") for fi in range(4)}
                for fi in range(4):
                    ensure_dq(dq_for_fi(fi))
                    for kd in range(KD):
                        nc.tensor.matmul(
                            a[fi],
                            wr_of[mat, kd][:, fg, fi * P:(fi + 1) * P],
                            xs(kd), start=(kd == 0), stop=(kd == KD - 1))
                    epi(fi, a[fi])

            gu_pass("g", 1, lambda fi: 16 + 4 * fi + 3,
                    lambda fi, ac: nc.scalar.activation(h_sb[:, 4 + fi], ac, silu))
            gu_pass("u", 0, lambda fi: 32 + 2 * fi + 1,
                    lambda fi, ac: nc.vector.tensor_mul(
                        out=h_sb[:, fi], in0=h_sb[:, fi], in1=ac))
            gu_pass("u", 1, lambda fi: 40 + 2 * fi + 1,
                    lambda fi, ac: nc.vector.tensor_mul(
                        out=h_sb[:, 4 + fi], in0=h_sb[:, 4 + fi], in1=ac))
            ensure_dq(47)

            # ---- passes 4-7: down mg, k-contiguous runs over kf ----
            for mg in range(4):
                a = {fi: psacc.tile([P, S], f32, name=f"acc{fi}",
                                    tag=f"acc{fi}") for fi in range(4)}
                for fi in range(4):
                    for kf in range(KF):
                        nc.tensor.matmul(
                            a[fi],
                            dwr_tiles[kf][:, mg, fi * P:(fi + 1) * P],
                            h_sb[:, kf], start=(kf == 0), stop=(kf == KF - 1))
                    ot = opool.tile([P, S], bf16, name="ot", tag="ot")
                    if fi % 2 == 0:
                        nc.scalar.copy(ot, a[fi])
                    else:
                        nc.vector.tensor_copy(out=ot, in_=a[fi])
                    weng = nc.sync if fi % 2 == 0 else nc.scalar
                    weng.dma_start(
                        out[(mg * 4 + fi) * P:(mg * 4 + fi + 1) * P, :], ot)
    nc.finalize()
    return nc


def _prep_inputs(x, gate_snapped, gate_scale_A, gate_scale_B,
                 up_snapped, up_scale_A, up_scale_B,
                 down_snapped, down_scale_A, down_scale_B):
    bf = ml_dtypes.bfloat16
    f = lambda a: np.asarray(a, dtype=np.float32)
    x2 = np.ascontiguousarray(f(x).reshape(D, S).astype(bf))
    gT_full = f(gate_snapped).T      # [D, FF] view
    uT_full = f(up_snapped).T
    dT_full = f(down_snapped).T      # [FF, D] view

    def pack_B4(Bmat, nk):
        # [R, nk*128] fp32 -> [128, nk/2, 128]: strips (0,1)=chunk 2j,
        # strips (2,3)=chunk 2j+1
        b = f(Bmat).reshape(R, nk // 2, 2, P).astype(bf)
        o = np.empty((4 * R, nk // 2, P), dtype=bf)
        o[0 * R:1 * R] = b[:, :, 0, :]
        o[1 * R:2 * R] = b[:, :, 0, :]
        o[2 * R:3 * R] = b[:, :, 1, :]
        o[3 * R:4 * R] = b[:, :, 1, :]
        return o

    def pack_AT4(Amat):
        # A [w, R] -> A^T [R, w] replicated on four strips -> [128, w]
        at = f(Amat).T.astype(bf)
        return np.ascontiguousarray(np.concatenate([at] * 4, axis=0))

    gB_f, uB_f, dB_f = f(gate_scale_B), f(up_scale_B), f(down_scale_B)
    gA_f, uA_f = f(gate_scale_A), f(up_scale_A)
    dAT4 = pack_AT4(down_scale_A)      # [128, D], same for all cores

    in_maps = []
    for c in range(NCORES):
        lo, hi = c * F, (c + 1) * F
        in_maps.append({
            "x": x2,
            "gT": np.ascontiguousarray(gT_full[:, lo:hi]).astype(bf)
                    .reshape(KD, P, 2, 512),
            "uT": np.ascontiguousarray(uT_full[:, lo:hi]).astype(bf)
                    .reshape(KD, P, 2, 512),
            "dT": np.ascontiguousarray(dT_full[lo:hi, :]).astype(bf)
                    .reshape(KF, P, 4, 512),
            "gB4": pack_B4(gB_f, KD),
            "uB4": pack_B4(uB_f, KD),
            "dB4": pack_B4(dB_f[:, lo:hi], KF),
            "gAT4": pack_AT4(gA_f[lo:hi]),
            "uAT4": pack_AT4(uA_f[lo:hi]),
            "dAT4": dAT4,
        })
    return in_maps


def run(trace=False, **inputs):
    if "nc" not in _CACHE:
        _CACHE["nc"] = _build()
    nc = _CACHE["nc"]
    in_maps = _prep_inputs(**inputs)
    try:
        res = run_bass_kernel_spmd(nc, in_maps, list(range(NCORES)), trace=trace)
    except Exception:
        # A transient device flake (NRT_EXEC_UNIT_UNRECOVERABLE) poisons the
        # PJRT client for the process; tearing the backend down and
        # reconnecting recovers it the same way a fresh process does.
        try:
            import jax.extend.backend
            jax.extend.backend.clear_backends()
        except Exception:
            pass
        res = run_bass_kernel_spmd(nc, in_maps, list(range(NCORES)), trace=trace)
    partial = np.zeros((D, S), dtype=np.float32)
    for c in range(NCORES):
        partial += np.asarray(res.results[c]["out"], dtype=np.float32)
    return partial.reshape(1, D, 1, S), res


def kernel(**inputs):
    out, _ = run(trace=False, **inputs)
    return out


if __name__ == "__main__":
    rng = np.random.default_rng(0)
    ins = {
        "x": rng.standard_normal((1, D, 1, S)).astype(np.float32),
        "gate_snapped": (rng.standard_normal((FF, D)) * 0.02).astype(np.float32),
        "gate_scale_A": (rng.standard_normal((FF, R)) * 0.1).astype(np.float32),
        "gate_scale_B": (rng.standard_normal((R, D)) * 0.1).astype(np.float32),
        "up_snapped": (rng.standard_normal((FF, D)) * 0.02).astype(np.float32),
        "up_scale_A": (rng.standard_normal((FF, R)) * 0.1).astype(np.float32),
        "up_scale_B": (rng.standard_normal((R, D)) * 0.1).astype(np.float32),
        "down_snapped": (rng.standard_normal((D, FF)) * 0.02).astype(np.float32),
        "down_scale_A": (rng.standard_normal((D, R)) * 0.1).astype(np.float32),
        "down_scale_B": (rng.standard_normal((R, FF)) * 0.1).astype(np.float32),
    }
    out = kernel(**ins)
    print("kernel ran, out shape", out.shape, "mean abs", np.abs(out).mean())


# revision 11
# speedup vs baseline: 1.2135x; 1.1630x over previous
"""Trainium2 Bass kernel for FFNWithScales (SwiGLU MLP with low-rank dequant scales).

Reference computation (all fp32):
    gate_eff = gate_snapped * (gate_scale_A @ gate_scale_B)       # [8192, 2048]
    up_eff   = up_snapped   * (up_scale_A   @ up_scale_B)         # [8192, 2048]
    down_eff = down_snapped * (down_scale_A @ down_scale_B)       # [2048, 8192]
    h   = silu(gate_eff @ x) * (up_eff @ x)                       # [8192, 512]
    out = down_eff @ h                                            # [2048, 512]

Sharding (8 cores, tensor-parallel on d_ff): core c owns d_ff rows
[c*1024, (c+1)*1024) of gate/up (and the matching columns of down).
Each core computes a full-[2048, 512] partial of the down projection;
partials are summed on the host (the all-reduce step).

Kernel design (v2 — PE-bound, so everything serves the PE stream):
  - All tensors ship bf16 from the host (snapped weights included): the
    extra bf16 rounding of snapped costs ~1e-3 relative error against a
    2e-2 budget, and it halves HBM traffic so DMA (~17 MB @ ~300 GB/s)
    stays far under the PE streaming time.
  - Weights are pre-transposed on host so their contraction dim rides the
    partitions, and are DMA'd in full-row tiles (2-4 KB contiguous per
    partition line): gate/up as [128 d, 1024 f] per d-chunk, down as
    [128 f, 2048 d] per f-chunk.
  - The rank-32 scale products run 4-way row-packed on the PE
    (tile_position strips 0/32/64/96): one ~280 ns stream covers two
    d-chunks' worth of scale tiles. The DVE dequant-multiplies the bf16
    snapped tile by the fp32 psum scale tile, emitting the bf16 wr tile
    the main matmuls consume. Dequanted gate/up/down weights stay
    resident in SBUF so each of the 48 dequants serves two passes.
  - Main matmuls are kd-major in the first pass (matches the dequant
    feed rate) and fi-major-blocked afterwards, so a psum accumulator's
    epilogue (silu / up-multiply / output copy) always drains behind
    12+ matmuls on other banks — pass boundaries never stall the PE.
  - Six dummy warm-up matmuls on a memset tile run while the first DMAs
    land, so the PE HAM clock-gate reaches 2.4 GHz before real work.
  - Output partials store bf16 (host accumulates in fp32), with the
    final pass's stores split across both HWDGE rings for a short tail.
"""

import numpy as np
import ml_dtypes

import concourse.bass as bass
from concourse import bacc
import concourse.mybir as mybir
from concourse.tile import TileContext
from concourse.bass_utils import run_bass_kernel_spmd

P = 128
D = 2048        # d_model
FF = 8192       # d_ff (global)
S = 512         # sequence
R = 32          # rank
NCORES = 8
F = FF // NCORES          # 1024 local d_ff rows
KD = D // P               # 16 d_model chunks
KF = F // P               # 8 local d_ff chunks

f32 = mybir.dt.float32
bf16 = mybir.dt.bfloat16

_CACHE = {}


def _build():
    nc = bacc.Bacc()
    x = nc.declare_dram_parameter("x", [D, S], bf16, isOutput=False)
    # snapped weights, transposed, tiled so a dram slice is an SBUF tile
    gT = nc.declare_dram_parameter("gT", [KD, P, 2, 512], bf16, isOutput=False)
    uT = nc.declare_dram_parameter("uT", [KD, P, 2, 512], bf16, isOutput=False)
    dT = nc.declare_dram_parameter("dT", [KF, P, 4, 512], bf16, isOutput=False)
    # 4-way packed scale factors: B strips for chunk pairs (both fg copies),
    # A^T replicated on all four 32-row strips.
    gB4 = nc.declare_dram_parameter("gB4", [4 * R, KD // 2, P], bf16, isOutput=False)
    uB4 = nc.declare_dram_parameter("uB4", [4 * R, KD // 2, P], bf16, isOutput=False)
    dB4 = nc.declare_dram_parameter("dB4", [4 * R, KF // 2, P], bf16, isOutput=False)
    gAT4 = nc.declare_dram_parameter("gAT4", [4 * R, F], bf16, isOutput=False)
    uAT4 = nc.declare_dram_parameter("uAT4", [4 * R, F], bf16, isOutput=False)
    dAT4 = nc.declare_dram_parameter("dAT4", [4 * R, D], bf16, isOutput=False)
    out = nc.declare_dram_parameter("out", [D, S], bf16, isOutput=True)

    silu = mybir.ActivationFunctionType.Silu

    with TileContext(nc) as tc:
        with (
            tc.tile_pool(name="const", bufs=1) as const,
            tc.tile_pool(name="wtg", bufs=6) as wtg,
            tc.tile_pool(name="wtd", bufs=8) as wtd,
            tc.tile_pool(name="gwr", bufs=1) as gwr,
            tc.tile_pool(name="uwr", bufs=1) as uwr,
            tc.tile_pool(name="dwr", bufs=1) as dwr,
            tc.tile_pool(name="hbuf", bufs=1) as hpool,
            tc.tile_pool(name="scb", bufs=3) as scbp,
            tc.tile_pool(name="obuf", bufs=3) as opool,
            tc.tile_pool(name="psacc", bufs=1, space="PSUM") as psacc,
            tc.tile_pool(name="pssc", bufs=2, space="PSUM") as pssc,
        ):
            # ---- constant loads (factors lead the rings, x0/x1 next) ----
            rounded = {}

            def load_const(nm, dram, eng):
                rt = const.tile(list(dram.shape), bf16, name=nm, tag=nm)
                eng.dma_start(rt, dram[:])
                rounded[nm] = rt

            load_const("gB4", gB4, nc.sync)
            load_const("gAT4", gAT4, nc.sync)

            x_sb = [None] * (KD // 2)

            def load_x_chunk(q, eng):
                xt = const.tile([P, 2, S], bf16, name=f"x{q}", tag=f"x{q}")
                eng.dma_start(
                    xt, x[q * 2 * P:(q + 1) * 2 * P, :].rearrange(
                        "(ko p) s -> p ko s", p=P))
                x_sb[q] = xt

            def xs(kd):
                return x_sb[kd // 2][:, kd % 2]

            load_x_chunk(0, nc.scalar)
            load_x_chunk(1, nc.scalar)

            # ---- PE warm-up: dummy matmuls while the first DMAs land ----
            junk = const.tile([P, 640], bf16, name="junk", tag="junk")
            nc.vector.memset(junk, 0.0)
            for _ in range(12):
                wps = pssc.tile([P, 2, S], f32, name="sc", tag="sc")
                nc.tensor.matmul(wps[:, 0], junk[:, 0:128], junk[:, 128:640],
                                 start=True, stop=True)

            # ---- weight stream: 40 snapped-tile DMAs ----
            # jobs 0..15 gate kd, 16..31 up kd, 32..39 down kf
            wt_tiles = {}
            dma_parity = [0]

            def ring():
                dma_parity[0] ^= 1
                return nc.sync if dma_parity[0] else nc.scalar

            def emit_wt(j):
                if j < 16:
                    t = wtg.tile([P, 2, 512], bf16, name="wt", tag="wt")
                    ring().dma_start(t, gT[j])
                elif j < 32:
                    t = wtg.tile([P, 2, 512], bf16, name="wt", tag="wt")
                    ring().dma_start(t, uT[j - 16])
                else:
                    t = wtd.tile([P, 4, 512], bf16, name="wtd", tag="wtd")
                    ring().dma_start(t, dT[j - 32])
                wt_tiles[j] = t

            # ---- scale bursts + copies + dequants ----
            # dq jobs: 0..15 gate kd, 16..31 up kd, 32..47 down (h*8 + kf)
            # burst b covers dq jobs 2b, 2b+1. Each job is a 3-engine chain:
            # PE scale-matmul -> ACT psum->sbuf bf16 copy -> DVE bf16 2x
            # dequant multiply (all-16-bit keeps the DVE in its fast mode).
            sc_tiles = {}
            scb_tiles = {}
            dwr_tiles = {}
            dq_next = [0]
            wt_next = [0]

            def ensure_wt(upto):
                while wt_next[0] <= min(upto, 39):
                    emit_wt(wt_next[0])
                    wt_next[0] += 1

            def emit_burst(b):
                sc_a = pssc.tile([P, 2, S], f32, name="sc", tag="sc")
                sc_b = pssc.tile([P, 2, S], f32, name="sc", tag="sc")
                if b < 8:          # gate kd pair (2b, 2b+1)
                    Bm, Am, j = rounded["gB4"], rounded["gAT4"], b
                    cols = [0, 512, 0, 512]
                elif b < 16:       # up kd pair
                    Bm, Am, j = rounded["uB4"], rounded["uAT4"], b - 8
                    cols = [0, 512, 0, 512]
                else:              # down: m = b-16: h = m//4, kf pair j = m%4
                    m = b - 16
                    h = m // 4
                    Bm, Am, j = rounded["dB4"], rounded["dAT4"], m % 4
                    cols = [h * 1024, h * 1024 + 512, h * 1024, h * 1024 + 512]
                for i, dst in enumerate((sc_a[:, 0], sc_a[:, 1],
                                         sc_b[:, 0], sc_b[:, 1])):
                    nc.tensor.matmul(
                        dst,
                        Bm[i * R:(i + 1) * R, j],
                        Am[i * R:(i + 1) * R, cols[i]:cols[i] + 512],
                        start=True, stop=True,
                        tile_position=(R * i, 0),
                    )
                sc_tiles[2 * b] = sc_a
                sc_tiles[2 * b + 1] = sc_b

            wr_of = {}

            def emit_dq(j):
                # three chain flavours, balanced across DVE/ACT/GPSIMD so no
                # single engine's backlog can stall the PE's sc-slot ring:
                #   j%4 in (0,2): DVE multiplies straight off the sc psum
                #   j%4 == 1:     ACT copies psum->sbuf, DVE multiplies (2x)
                #   j%4 == 3:     ACT copies psum->sbuf, GPSIMD multiplies
                sc = sc_tiles.pop(j)
                if j % 2 == 1:
                    scb = scbp.tile([P, 2, 512], bf16, name="scb", tag="scb")
                    nc.scalar.copy(scb, sc)
                    sc = scb
                    meng = nc.vector if j % 4 == 1 else nc.gpsimd
                else:
                    meng = nc.vector
                if j < 16:
                    wr = gwr.tile([P, 2, 512], bf16, name="gwr", tag=f"g{j}")
                    meng.tensor_mul(out=wr, in0=wt_tiles.pop(j), in1=sc)
                    wr_of["g", j] = wr
                elif j < 32:
                    wr = uwr.tile([P, 2, 512], bf16, name="uwr", tag=f"u{j-16}")
                    meng.tensor_mul(out=wr, in0=wt_tiles.pop(j), in1=sc)
                    wr_of["u", j - 16] = wr
                else:
                    m = j - 32
                    h, kf = m // 8, m % 8
                    if h == 0:
                        dwr_tiles[kf] = dwr.tile([P, 4, 512], bf16, name="dwr",
                                                 tag=f"d{kf}")
                    wr = dwr_tiles[kf]
                    wt = wt_tiles[32 + kf] if h == 0 else wt_tiles.pop(32 + kf)
                    meng.tensor_mul(out=wr[:, 2 * h:2 * h + 2],
                                    in0=wt[:, 2 * h:2 * h + 2], in1=sc)
                    wr_of["d", kf] = wr

            def _wt_for_dq(j):
                return j if j < 32 else 32 + (j - 32) % 8

            def ensure_dq(upto):
                while dq_next[0] <= min(upto, 47):
                    jj = dq_next[0]
                    ensure_wt(_wt_for_dq(jj) + 4)
                    if jj % 2 == 0:
                        emit_burst(jj // 2)
                    emit_dq(jj)
                    dq_next[0] += 1

            # prime the pipeline; up/down factors ride the HWDGE rings right
            # behind the first weight tiles (needed ~25 us in, land ~15)
            ensure_wt(3)
            for nm, dram in (("uB4", uB4), ("uAT4", uAT4),
                             ("dB4", dB4), ("dAT4", dAT4)):
                load_const(nm, dram, ring())
            ensure_dq(1)

            h_sb = hpool.tile([P, KF, S], bf16)

            # ---- pass 0: gate fg0, kd-pair-major (dequant-feed limited;
            # psum banks switch every 2 matmuls instead of every 1) ----
            acc = {fi: psacc.tile([P, S], f32, name=f"acc{fi}", tag=f"acc{fi}")
                   for fi in range(4)}
            for kdp in range(KD // 2):
                ensure_dq(min(2 * kdp + 3, 15))
                if kdp <= 5:
                    load_x_chunk(kdp + 2, ring())
                for fi in range(4):
                    for kd in (2 * kdp, 2 * kdp + 1):
                        nc.tensor.matmul(
                            acc[fi], wr_of["g", kd][:, 0, fi * P:(fi + 1) * P],
                            xs(kd), start=(kd == 0), stop=(kd == KD - 1))
            for fi in range(4):
                nc.scalar.activation(h_sb[:, fi], acc[fi], silu)

            # ---- passes 1-3: gate fg1 / up fg0 / up fg1 ----
            # Full k-contiguous runs: 16 matmuls into ONE psum bank (the
            # b2b rate is 216 ns same-bank vs 259 ns cycling), epilogue of
            # acc[fi] drains behind acc[fi+1]'s run.
            def gu_pass(mat, fg, dq_for_fi, epi):
                a = {fi: psacc.tile([P, S], f32, name=f"acc{fi}",
                                    tag=f"acc{fi}") for fi in range(4)}
                for fi in range(4):
                    ensure_dq(dq_for_fi(fi))
                    for kd in range(KD):
                        nc.tensor.matmul(
                            a[fi],
                            wr_of[mat, kd][:, fg, fi * P:(fi + 1) * P],
                            xs(kd), start=(kd == 0), stop=(kd == KD - 1))
                    epi(fi, a[fi])

            gu_pass("g", 1, lambda fi: 16 + 4 * fi + 3,
                    lambda fi, ac: nc.scalar.activation(h_sb[:, 4 + fi], ac, silu))
            gu_pass("u", 0, lambda fi: 32 + 2 * fi + 1,
                    lambda fi, ac: nc.vector.tensor_mul(
                        out=h_sb[:, fi], in0=h_sb[:, fi], in1=ac))
            gu_pass("u", 1, lambda fi: 40 + 2 * fi + 1,
                    lambda fi, ac: nc.vector.tensor_mul(
                        out=h_sb[:, 4 + fi], in0=h_sb[:, 4 + fi], in1=ac))
            ensure_dq(47)

            # ---- passes 4-7: down mg, k-contiguous runs over kf ----
            for mg in range(4):
                a = {fi: psacc.tile([P, S], f32, name=f"acc{fi}",
                                    tag=f"acc{fi}") for fi in range(4)}
                for fi in range(4):
                    for kf in range(KF):
                        nc.tensor.matmul(
                            a[fi],
                            dwr_tiles[kf][:, mg, fi * P:(fi + 1) * P],
                            h_sb[:, kf], start=(kf == 0), stop=(kf == KF - 1))
                    ot = opool.tile([P, S], bf16, name="ot", tag="ot")
                    if fi % 2 == 0:
                        nc.scalar.copy(ot, a[fi])
                    else:
                        nc.vector.tensor_copy(out=ot, in_=a[fi])
                    weng = nc.sync if fi % 2 == 0 else nc.scalar
                    weng.dma_start(
                        out[(mg * 4 + fi) * P:(mg * 4 + fi + 1) * P, :], ot)
    nc.finalize()
    return nc


def _prep_inputs(x, gate_snapped, gate_scale_A, gate_scale_B,
                 up_snapped, up_scale_A, up_scale_B,
                 down_snapped, down_scale_A, down_scale_B):
    bf = ml_dtypes.bfloat16
    f = lambda a: np.asarray(a, dtype=np.float32)
    x2 = np.ascontiguousarray(f(x).reshape(D, S).astype(bf))
    gT_full = f(gate_snapped).T      # [D, FF] view
    uT_full = f(up_snapped).T
    dT_full = f(down_snapped).T      # [FF, D] view

    def pack_B4(Bmat, nk):
        # [R, nk*128] fp32 -> [128, nk/2, 128]: strips (0,1)=chunk 2j,
        # strips (2,3)=chunk 2j+1
        b = f(Bmat).reshape(R, nk // 2, 2, P).astype(bf)
        o = np.empty((4 * R, nk // 2, P), dtype=bf)
        o[0 * R:1 * R] = b[:, :, 0, :]
        o[1 * R:2 * R] = b[:, :, 0, :]
        o[2 * R:3 * R] = b[:, :, 1, :]
        o[3 * R:4 * R] = b[:, :, 1, :]
        return o

    def pack_AT4(Amat):
        # A [w, R] -> A^T [R, w] replicated on four strips -> [128, w]
        at = f(Amat).T.astype(bf)
        return np.ascontiguousarray(np.concatenate([at] * 4, axis=0))

    gB_f, uB_f, dB_f = f(gate_scale_B), f(up_scale_B), f(down_scale_B)
    gA_f, uA_f = f(gate_scale_A), f(up_scale_A)
    dAT4 = pack_AT4(down_scale_A)      # [128, D], same for all cores

    in_maps = []
    for c in range(NCORES):
        lo, hi = c * F, (c + 1) * F
        in_maps.append({
            "x": x2,
            "gT": np.ascontiguousarray(gT_full[:, lo:hi]).astype(bf)
                    .reshape(KD, P, 2, 512),
            "uT": np.ascontiguousarray(uT_full[:, lo:hi]).astype(bf)
                    .reshape(KD, P, 2, 512),
            "dT": np.ascontiguousarray(dT_full[lo:hi, :]).astype(bf)
                    .reshape(KF, P, 4, 512),
            "gB4": pack_B4(gB_f, KD),
            "uB4": pack_B4(uB_f, KD),
            "dB4": pack_B4(dB_f[:, lo:hi], KF),
            "gAT4": pack_AT4(gA_f[lo:hi]),
            "uAT4": pack_AT4(uA_f[lo:hi]),
            "dAT4": dAT4,
        })
    return in_maps


def run(trace=False, **inputs):
    if "nc" not in _CACHE:
        _CACHE["nc"] = _build()
    nc = _CACHE["nc"]
    in_maps = _prep_inputs(**inputs)
    try:
        res = run_bass_kernel_spmd(nc, in_maps, list(range(NCORES)), trace=trace)
    except Exception:
        # A transient device flake (NRT_EXEC_UNIT_UNRECOVERABLE) poisons the
        # PJRT client for the process; tearing the backend down and
        # reconnecting recovers it the same way a fresh process does.
        try:
            import jax.extend.backend
            jax.extend.backend.clear_backends()
        except Exception:
            pass
        res = run_bass_kernel_spmd(nc, in_maps, list(range(NCORES)), trace=trace)
    partial = np.zeros((D, S), dtype=np.float32)
    for c in range(NCORES):
        partial += np.asarray(res.results[c]["out"], dtype=np.float32)
    return partial.reshape(1, D, 1, S), res


def kernel(**inputs):
    out, _ = run(trace=False, **inputs)
    return out


if __name__ == "__main__":
    rng = np.random.default_rng(0)
    ins = {
        "x": rng.standard_normal((1, D, 1, S)).astype(np.float32),
        "gate_snapped": (rng.standard_normal((FF, D)) * 0.02).astype(np.float32),
        "gate_scale_A": (rng.standard_normal((FF, R)) * 0.1).astype(np.float32),
        "gate_scale_B": (rng.standard_normal((R, D)) * 0.1).astype(np.float32),
        "up_snapped": (rng.standard_normal((FF, D)) * 0.02).astype(np.float32),
        "up_scale_A": (rng.standard_normal((FF, R)) * 0.1).astype(np.float32),
        "up_scale_B": (rng.standard_normal((R, D)) * 0.1).astype(np.float32),
        "down_snapped": (rng.standard_normal((D, FF)) * 0.02).astype(np.float32),
        "down_scale_A": (rng.standard_normal((D, R)) * 0.1).astype(np.float32),
        "down_scale_B": (rng.standard_normal((R, FF)) * 0.1).astype(np.float32),
    }
    out = kernel(**ins)
    print("kernel ran, out shape", out.shape, "mean abs", np.abs(out).mean())
